# revision 11
# baseline (speedup 1.0000x reference)
"""MemoryReader attention kernel for 8x Trainium2 NeuronCores.

Computation per batch (B=16, CK=64, CV=512, N=HW=3136):
    scores[n, m] = (2 * mk_f[:,n]@qk_f[:,m] - ||mk_f[:,n]||^2) / sqrt(CK)
    A = softmax(scores, axis=n)
    mem[c, m]  = sum_n mv_f[c, n] * A[n, m]
    outputs: (mem, qv)   -- qv is a pure passthrough (host-side).

Sharding: data-parallel over batch, 2 batches per core.

Layout: memory dim n on PSUM partitions (25 tiles of 128), query dim m on
the free axis (8 chunks of 392).

v2 design notes:
  * mk/qk are staged to bf16 [64, N]; the scores matmul runs K=64 bf16
    (full PE rate).  The -||mk||^2/8 term is applied as a per-partition
    f32 bias in the ScalarE exp (exact), computed once per batch by 25
    one-column matmuls (msq^T @ ones).
  * E = exp(scores) is written in bf16 (same PE rate as f32r for the
    readout moving operand, half the SBUF, 2x DVE/Pool read rate).
  * The softmax denominator is mostly accumulated on the Pool engine
    (esum += E[t], tiles 0..22); the PE only does ones^T@esum plus two
    correction matmuls for tiles 23/24 and the K=1 reciprocal broadcast.
  * mv is DMAed whole (contiguous 12.5KB lines), PE-transposed in f32r
    (1.5 cyc/row), and converted to bf16 mvt by DVE.
  * PSUM evacuation is fused with the softmax normalization: DVE does
    c=0,1 and Pool does c=2,3 of out = mem_psum * (1/den) directly from
    PSUM.  ScalarE runs only exp/ln (one activation table, no reloads).
  * mm2 (readout) trails mm1 (scores) by 8 tiles so the denominator
    chain at the chunk boundary hides under PE work; s-PSUM is double
    buffered (banks: 2 scores + 4 readout + 1 row + 1 misc = 8).
  * The next batch's staging (mk/qk DMA+convert, bias matmuls, mv
    DMA+transpose) is emitted inside the current batch's chunk loop so
    batch startup overlaps compute; all cross-batch pools have bufs=2.
"""
import sys

if "/opt/trn_rl_repo" not in sys.path:
    sys.path.insert(0, "/opt/trn_rl_repo")

import numpy as np

import concourse.bacc as bacc
import concourse.mybir as mybir
import concourse.tile as tile
from concourse.bass_utils import run_bass_kernel_spmd

F32 = mybir.dt.float32
F32R = mybir.dt.float32r
BF16 = mybir.dt.bfloat16
EXP = mybir.ActivationFunctionType.Exp
LN = mybir.ActivationFunctionType.Ln

B, CK, CV, H, W = 16, 64, 512, 56, 56
N = H * W                      # 3136: memory positions == query positions
NB = 2                         # batches per core
NCORES = 8
NT = 25                        # n tiles: 24*128 + 64
CH = 8                         # m chunks
MC = N // CH                   # 392
CT = CV // 128                 # 4 c tiles
LAG = 8                        # mm2 trails mm1 by this many n-tiles


def _ntile(t):
    return (t * 128, 64 if t == NT - 1 else 128)


def build_nc(rep=1, loop=0):
    nc = bacc.Bacc("TRN2", target_bir_lowering=False, debug=False,
                   num_devices=NCORES)
    mk_d = nc.dram_tensor("mk", [NB, CK, N], F32, kind="ExternalInput")
    qk_d = nc.dram_tensor("qk", [NB, CK, N], F32, kind="ExternalInput")
    mv_d = nc.dram_tensor("mv", [NB, CV, N], F32R, kind="ExternalInput")
    id_d = nc.dram_tensor("ident", [128, 128], F32R, kind="ExternalInput")
    out_d = nc.dram_tensor("out", [NB, CV, N], F32, kind="ExternalOutput")

    batches = [b for _ in range(rep) for b in range(NB)]
    NBAT = len(batches)

    with tile.TileContext(nc) as tc:
        with (
            tc.tile_pool(name="const", bufs=1) as constp,
            tc.tile_pool(name="stage", bufs=2) as stagep,
            tc.tile_pool(name="keys", bufs=2) as keysp,
            tc.tile_pool(name="msq", bufs=1) as msqp,
            tc.tile_pool(name="mvstage", bufs=1) as mvstagep,
            tc.tile_pool(name="mvt", bufs=2) as mvtp,
            tc.tile_pool(name="asb", bufs=2) as asbp,
            tc.tile_pool(name="ebuf", bufs=2) as ep,
            tc.tile_pool(name="esum", bufs=2) as esump,
            tc.tile_pool(name="osb", bufs=2) as outp,
            tc.tile_pool(name="bcsb", bufs=2) as bcp,
            tc.tile_pool(name="row", bufs=1) as rowp,
            tc.tile_pool(name="ps_s", bufs=2, space="PSUM") as ps_s,
            tc.tile_pool(name="ps_mem", bufs=1, space="PSUM") as ps_mem,
            tc.tile_pool(name="ps_cs", bufs=1, space="PSUM") as ps_cs,
            tc.tile_pool(name="ps_tr", bufs=1, space="PSUM") as ps_tr,
        ):
            # ---- constants ----
            ident = constp.tile([128, 128], F32R)
            nc.sync.dma_start(ident[:], id_d[:])
            onesf = constp.tile([128, 1], F32)
            nc.gpsimd.memset(onesf[:], 1.0)
            ones128 = constp.tile([128, 2], F32R)
            nc.vector.tensor_copy(ones128[:, 0:1], onesf[:])
            nc.vector.tensor_copy(ones128[:, 1:2], onesf[:])
            ones64 = constp.tile([64, 2], F32R)
            nc.vector.tensor_copy(ones64[:, 0:1], onesf[0:64, :])
            nc.vector.tensor_copy(ones64[:, 1:2], onesf[0:64, :])
            ones16 = constp.tile([128, 2], BF16)
            nc.vector.tensor_copy(ones16[:, 0:1], onesf[:])
            nc.vector.tensor_copy(ones16[:, 1:2], onesf[:])
            onesbf = constp.tile([1, 128], F32)
            nc.gpsimd.memset(onesbf[:], 1.0)
            onesb = constp.tile([1, 128], F32R)
            nc.vector.tensor_copy(onesb[:], onesbf[:])

            # ---- per-batch startup pieces ----
            def emit_staging(b):
                """mk/qk DMA + bf16 convert + msq; mv DMA (4 quarters)."""
                mkb = keysp.tile([CK, N], BF16, tag="mkb")
                qkb = keysp.tile([CK, N], BF16, tag="qkb")
                msq = msqp.tile([CK, N], F32R, tag="msq")
                for u in range(CH):
                    sl = slice(u * MC, (u + 1) * MC)
                    st = stagep.tile([CK, MC], F32, tag="stmk")
                    nc.sync.dma_start(st[:], mk_d[b, :, sl])
                    nc.vector.tensor_copy(mkb[:, sl], st[:])
                    nc.vector.tensor_mul(msq[:, sl], st[:], st[:])
                    st2 = stagep.tile([CK, MC], F32, tag="stqk")
                    nc.sync.dma_start(st2[:], qk_d[b, :, sl])
                    nc.vector.tensor_copy(qkb[:, sl], st2[:])
                mvs = mvstagep.tile([128, CT, N], F32R, tag="mvs")
                mv_r = mv_d[b].rearrange("(ct p) n -> p ct n", p=128)
                NQ = N // 4
                for q in range(4):
                    qs = slice(q * NQ, (q + 1) * NQ)
                    nc.sync.dma_start(mvs[:, :, qs], mv_r[:, :, qs])
                return {"mkb": mkb, "qkb": qkb, "msq": msq, "mvs": mvs}

            def emit_bias(st):
                """a_sb[p, t] = -||mk_{128t+p}||^2 / 8 via 25 tiny matmuls."""
                a_ps = ps_tr.tile([128, 512], F32, tag="tr")
                for t in range(NT):
                    n0, pt = _ntile(t)
                    nc.tensor.matmul(a_ps[0:pt, 2 * t:2 * t + 2],
                                     st["msq"][:, n0:n0 + pt], ones64[:],
                                     start=True, stop=True)
                a_sb = asbp.tile([128, 64], F32, tag="asb")
                nc.vector.tensor_scalar_mul(a_sb[:, 0:2 * NT - 2],
                                            a_ps[:, 0:2 * NT - 2], -0.125)
                nc.vector.tensor_scalar_mul(a_sb[0:64, 2 * NT - 2:2 * NT],
                                            a_ps[0:64, 2 * NT - 2:2 * NT],
                                            -0.125)
                st["a_sb"] = a_sb

            def emit_mvt(st):
                """mvt[n_p, t, c*128+c'] = mv[c*128+c', 128t+n_p] in bf16."""
                mvt = mvtp.tile([128, NT, 512], BF16, tag="mvt")
                for t in range(NT):
                    n0, pt = _ntile(t)
                    tr = ps_tr.tile([128, 512], F32R, tag="tr")
                    for c in range(CT):
                        nc.tensor.transpose(tr[0:pt, c * 128:(c + 1) * 128],
                                            st["mvs"][:, c, n0:n0 + pt],
                                            ident[:])
                    nc.vector.tensor_copy(mvt[0:pt, t, :], tr[0:pt, :])
                st["mvt"] = mvt

            def emit_startup(b):
                st = emit_staging(b)
                emit_bias(st)
                emit_mvt(st)
                return st

            # ---- main per-chunk pipeline ----
            def chunk(b, st, u, prefetch=None):
                sl = slice(u * MC, (u + 1) * MC)
                if prefetch is not None:
                    prefetch()
                E = ep.tile([128, NT, MC], BF16, tag="E")
                mem_ps = ps_mem.tile([128, CT, 512], F32, tag="mem")
                esum = esump.tile([128, MC], F32R, tag="esum")
                out_sb = outp.tile([128, CT, MC], F32, tag="osb")

                def mm1(t):
                    n0, pt = _ntile(t)
                    sp = ps_s.tile([128, 512], F32, tag="s")
                    nc.tensor.matmul(sp[0:pt, 0:MC], st["mkb"][:, n0:n0 + pt],
                                     st["qkb"][:, sl], start=True, stop=True)
                    nc.scalar.activation(E[0:pt, t, :], sp[0:pt, 0:MC], EXP,
                                         scale=0.25,
                                         bias=st["a_sb"][0:pt, 2 * t:2 * t + 1])
                    # DVE accumulates tiles 0..NT-3; 23/24 go via PE below.
                    if t == 0:
                        nc.vector.tensor_copy(esum[:], E[:, 0, :])
                    elif t <= NT - 3:
                        nc.vector.tensor_add(esum[:], esum[:], E[:, t, :])

                def mm2grp(t):
                    n0, pt = _ntile(t)
                    first, last = t == 0, t == NT - 1
                    for c in range(CT):
                        nc.tensor.matmul(mem_ps[:, c, 0:MC],
                                         st["mvt"][0:pt, t,
                                                   c * 128:(c + 1) * 128],
                                         E[0:pt, t, :],
                                         start=first, stop=last)

                for t in range(NT):
                    mm1(t)
                    if t >= LAG:
                        mm2grp(t - LAG)
                # drain mm2 with the denominator chain interleaved
                cs_ps = ps_cs.tile([2, 512], F32, tag="cs")
                rcf = rowp.tile([1, MC], F32, tag="rcf")
                rcr = rowp.tile([1, MC], F32R, tag="rc")
                for i, t in enumerate(range(NT - LAG, NT)):
                    mm2grp(t)
                    if i == 0:
                        nc.tensor.matmul(cs_ps[0:2, 0:MC], ones128[:],
                                         esum[:], start=True, stop=False)
                    elif i == 1:
                        nc.tensor.matmul(cs_ps[0:2, 0:MC], ones16[:],
                                         E[:, NT - 2, :],
                                         start=False, stop=False)
                    elif i == 2:
                        nc.tensor.matmul(cs_ps[0:2, 0:MC], ones16[0:64, :],
                                         E[0:64, NT - 1, :],
                                         start=False, stop=True)
                        nc.vector.reciprocal_approx_fast(rcf[:],
                                                         cs_ps[0:1, 0:MC])
                        nc.vector.tensor_copy(rcr[:], rcf[:])
                    elif i == 5:
                        bc_ps = ps_tr.tile([128, 512], F32, tag="tr")
                        nc.tensor.matmul(bc_ps[:, 0:MC], onesb[:], rcr[:],
                                         start=True, stop=True)
                        bc_sb = bcp.tile([128, MC], F32, tag="bc")
                        nc.vector.tensor_copy(bc_sb[:], bc_ps[:, 0:MC])
                # evacuate + normalize on DVE (Pool cannot access PSUM)
                for c in range(CT):
                    nc.vector.tensor_mul(out_sb[:, c, :], mem_ps[:, c, 0:MC],
                                         bc_sb[:])
                dst = out_d[b].rearrange("(ct p) n -> p ct n", p=128)
                nc.sync.dma_start(dst[:, :, sl], out_sb[:])

            def body():
                # Batch 0's startup is emitted at the body top: its DMAs
                # still prefetch across the For_i iteration boundary (they
                # only WAR on batch 0's previous readers, which finish mid
                # iteration), leaving only ~10us of DVE/PE staging work
                # serial at the boundary.  Batch i+1's startup is emitted
                # inside batch i's chunk loop so it overlaps compute.
                sts = {0: emit_startup(batches[0])}
                for i in range(NBAT):
                    for u in range(CH):
                        pf = None
                        if i + 1 < NBAT:
                            if u == 2:
                                pf = lambda j=i + 1: sts.__setitem__(
                                    j, emit_staging(batches[j]))
                            elif u == 4:
                                pf = lambda j=i + 1: emit_bias(sts[j])
                            elif u == 5:
                                pf = lambda j=i + 1: emit_mvt(sts[j])
                        chunk(batches[i], sts[i], u, pf)

            if loop:
                with tc.For_i(0, loop, 1):
                    body()
            else:
                body()

    nc.compile()
    return nc


_NC = None


def _get_nc():
    global _NC
    if _NC is None:
        _NC = build_nc()
    return _NC


def kernel(mk, qk, mv, qv):
    mk = np.ascontiguousarray(np.asarray(mk, dtype=np.float32)).reshape(B, CK, N)
    qk = np.ascontiguousarray(np.asarray(qk, dtype=np.float32)).reshape(B, CK, N)
    mv = np.ascontiguousarray(np.asarray(mv, dtype=np.float32)).reshape(B, CV, N)
    ident = np.eye(128, dtype=np.float32)

    nc = _get_nc()
    in_maps = [
        {"mk": mk[NB * i:NB * (i + 1)],
         "qk": qk[NB * i:NB * (i + 1)],
         "mv": mv[NB * i:NB * (i + 1)],
         "ident": ident}
        for i in range(NCORES)
    ]
    res = run_bass_kernel_spmd(nc, in_maps, list(range(NCORES)))
    mem = np.concatenate([res.results[i]["out"] for i in range(NCORES)], axis=0)
    mem = mem.reshape(B, CV, H, W)
    return mem, np.asarray(qv)


# revision 19
# speedup vs baseline: 1.1301x; 1.1301x over previous
"""MemoryReader attention kernel for 8x Trainium2 NeuronCores.

Computation per batch (B=16, CK=64, CV=512, N=HW=3136):
    scores[n, m] = (2 * mk_f[:,n]@qk_f[:,m] - ||mk_f[:,n]||^2) / sqrt(CK)
    A = softmax(scores, axis=n)
    mem[c, m]  = sum_n mv_f[c, n] * A[n, m]
    outputs: (mem, qv)   -- qv is a pure passthrough (host-side).

Sharding: data-parallel over batch, 2 batches per core.

Layout: memory dim n on PSUM partitions (25 tiles of 128), query dim m on
the free axis (7 chunks of 448).

Design notes (driven by HW probes; the CoreSim cost model underestimates
per-instruction overheads, so instruction count matters as much as rows):
  * mk/qk staged to bf16 [97, N]: rows 0..63 keys, row 64 = hi and row 96
    = lo bf16 split of -||mk||^2/2 (aug rows must start at 32-aligned
    partitions; rows 65..95 are zeroed filler).  Extra contraction rows
    are free (matmul cost is per output column), and this avoids a
    per-partition bias AP on the exp, which measures ~380ns/instruction
    extra on HW.
  * E = exp(0.25 * scores_mm) in bf16 via ScalarE straight out of PSUM.
  * Softmax denominator: two wide DVE tensor_reduce ops sum E tiles
    0..9 and 10..19 (emitted mid-chunk so they overlap the readout);
    PE accumulates only tiles 20..24 plus the two esum cross-partition
    reductions (7 small matmuls per chunk instead of 25).
  * Reciprocal via DVE reciprocal_approx_fast (no ScalarE Ln => no
    activation-table reloads), broadcast across partitions by a K=1
    matmul, applied fused with PSUM evacuation on DVE.
  * Output is written bf16 (halves output DMA; host upcasts).
  * mm2 (readout) trails mm1 (scores) by 8 tiles; s-PSUM double
    buffered (banks: 2 scores + 4 readout + 1 row + 1 misc = 8).
  * The next batch's staging (mk/qk DMA+convert+aug, mv DMA+transpose)
    is emitted inside the current batch's chunk loop so batch startup
    overlaps compute; cross-batch pools have bufs=2.
"""
import sys

if "/opt/trn_rl_repo" not in sys.path:
    sys.path.insert(0, "/opt/trn_rl_repo")

import numpy as np

import concourse.bacc as bacc
import concourse.mybir as mybir
import concourse.tile as tile
from concourse.bass_utils import run_bass_kernel_spmd

F32 = mybir.dt.float32
F32R = mybir.dt.float32r
BF16 = mybir.dt.bfloat16
EXP = mybir.ActivationFunctionType.Exp

B, CK, CV, H, W = 16, 64, 512, 56, 56
N = H * W                      # 3136
NB = 2                         # batches per core
NCORES = 8
NT = 25                        # n tiles: 24*128 + 64
CH = 7                         # m chunks
MC = N // CH                   # 448
CT = CV // 128                 # 4 c tiles
LAG = 8                        # mm2 trails mm1 by this many n-tiles
NRED = 10                      # tiles per DVE reduce (2 reduces: 0..19)
CSTART = 2 * NRED              # PE accumulates tiles CSTART..NT-1


def _ntile(t):
    return (t * 128, 64 if t == NT - 1 else 128)


def build_nc(rep=1, loop=0):
    nc = bacc.Bacc("TRN2", target_bir_lowering=False, debug=False,
                   num_devices=NCORES)
    mk_d = nc.dram_tensor("mk", [NB, CK, N], F32, kind="ExternalInput")
    qk_d = nc.dram_tensor("qk", [NB, CK, N], F32, kind="ExternalInput")
    mv_d = nc.dram_tensor("mv", [NB, CV, N], F32R, kind="ExternalInput")
    id_d = nc.dram_tensor("ident", [128, 128], F32R, kind="ExternalInput")
    out_d = nc.dram_tensor("out", [NB, CV, N], BF16, kind="ExternalOutput")

    batches = [b for _ in range(rep) for b in range(NB)]
    NBAT = len(batches)

    with tile.TileContext(nc) as tc:
        with (
            tc.tile_pool(name="const", bufs=1) as constp,
            tc.tile_pool(name="stage", bufs=2) as stagep,
            tc.tile_pool(name="keys", bufs=2) as keysp,
            tc.tile_pool(name="mvstage", bufs=1) as mvstagep,
            tc.tile_pool(name="mvt", bufs=2) as mvtp,
            tc.tile_pool(name="ebuf", bufs=2) as ep,
            tc.tile_pool(name="esum", bufs=2) as esump,
            tc.tile_pool(name="osb", bufs=2) as outp,
            tc.tile_pool(name="bcsb", bufs=2) as bcp,
            tc.tile_pool(name="row", bufs=1) as rowp,
            tc.tile_pool(name="ps_s", bufs=2, space="PSUM") as ps_s,
            tc.tile_pool(name="ps_mem", bufs=1, space="PSUM") as ps_mem,
            tc.tile_pool(name="ps_cs", bufs=1, space="PSUM") as ps_cs,
            tc.tile_pool(name="ps_tr", bufs=1, space="PSUM") as ps_tr,
        ):
            # ---- constants ----
            ident = constp.tile([128, 128], F32R)
            nc.sync.dma_start(ident[:], id_d[:])
            onesf = constp.tile([128, 1], F32)
            nc.gpsimd.memset(onesf[:], 1.0)
            ones64 = constp.tile([64, 2], F32R)
            nc.vector.tensor_copy(ones64[:, 0:1], onesf[0:64, :])
            nc.vector.tensor_copy(ones64[:, 1:2], onesf[0:64, :])
            ones16 = constp.tile([128, 2], BF16)
            nc.vector.tensor_copy(ones16[:, 0:1], onesf[:])
            nc.vector.tensor_copy(ones16[:, 1:2], onesf[:])
            ones128r = constp.tile([128, 2], F32R)
            nc.vector.tensor_copy(ones128r[:, 0:1], onesf[:])
            nc.vector.tensor_copy(ones128r[:, 1:2], onesf[:])
            onesbf = constp.tile([1, 128], F32)
            nc.gpsimd.memset(onesbf[:], 1.0)
            onesb = constp.tile([1, 128], F32R)
            nc.vector.tensor_copy(onesb[:], onesbf[:])

            # ---- per-batch startup pieces ----
            def emit_staging(b):
                """mk/qk DMA + bf16 convert; -||mk||^2/2 hi/lo aug rows at
                partitions 64/96; mv DMA (4 quarters)."""
                mkb = keysp.tile([97, N], BF16, tag="mkb")
                qkb = keysp.tile([97, N], BF16, tag="qkb")
                nc.gpsimd.memset(mkb[CK:96, :], 0.0)
                nc.gpsimd.memset(qkb[CK:96, :], 0.0)
                nc.gpsimd.memset(qkb[CK:CK + 1, :], 1.0)
                nc.gpsimd.memset(qkb[96:97, :], 1.0)
                for u in range(CH):
                    sl = slice(u * MC, (u + 1) * MC)
                    st = stagep.tile([CK, MC], F32, tag="stmk")
                    nc.sync.dma_start(st[:], mk_d[b, :, sl])
                    nc.vector.tensor_copy(mkb[0:CK, sl], st[:])
                    msq = stagep.tile([CK, MC], F32R, tag="msq")
                    nc.vector.tensor_mul(msq[:], st[:], st[:])
                    st2 = stagep.tile([CK, MC], F32, tag="stqk")
                    nc.sync.dma_start(st2[:], qk_d[b, :, sl])
                    nc.vector.tensor_copy(qkb[0:CK, sl], st2[:])
                    a_ps = ps_tr.tile([2, 512], F32, tag="tr")
                    nc.tensor.matmul(a_ps[0:2, 0:MC], ones64[:], msq[:],
                                     start=True, stop=True)
                    am = rowp.tile([1, MC], F32, tag="am")
                    nc.vector.tensor_scalar_mul(am[:], a_ps[0:1, 0:MC], -0.5)
                    h0 = rowp.tile([1, MC], BF16, tag="h0")
                    nc.vector.tensor_copy(h0[:], am[:])
                    am2 = rowp.tile([1, MC], F32, tag="am2")
                    nc.vector.tensor_sub(am2[:], am[:], h0[:])
                    nc.scalar.copy(mkb[CK:CK + 1, sl], h0[:])
                    nc.scalar.copy(mkb[96:97, sl], am2[:])
                mvs = mvstagep.tile([128, CT, N], F32R, tag="mvs")
                mv_r = mv_d[b].rearrange("(ct p) n -> p ct n", p=128)
                NQ = N // 4
                for q in range(4):
                    qs = slice(q * NQ, (q + 1) * NQ)
                    nc.sync.dma_start(mvs[:, :, qs], mv_r[:, :, qs])
                return {"mkb": mkb, "qkb": qkb, "mvs": mvs}

            def emit_mvt(st):
                """mvt[n_p, t, c*128+c'] = mv[c*128+c', 128t+n_p] in bf16."""
                mvt = mvtp.tile([128, NT, 512], BF16, tag="mvt")
                for t in range(NT):
                    n0, pt = _ntile(t)
                    tr = ps_tr.tile([128, 512], F32R, tag="tr")
                    for c in range(CT):
                        nc.tensor.transpose(tr[0:pt, c * 128:(c + 1) * 128],
                                            st["mvs"][:, c, n0:n0 + pt],
                                            ident[:])
                    nc.vector.tensor_copy(mvt[0:pt, t, :], tr[0:pt, :])
                st["mvt"] = mvt

            def emit_startup(b):
                st = emit_staging(b)
                emit_mvt(st)
                return st

            def mm2grp(t, st, mem_ps, E):
                n0, pt = _ntile(t)
                first, last = t == 0, t == NT - 1
                for c in range(CT):
                    nc.tensor.matmul(mem_ps[:, c, 0:MC],
                                     st["mvt"][0:pt, t, c * 128:(c + 1) * 128],
                                     E[0:pt, t, :],
                                     start=first, stop=last)

            # ---- main per-chunk pipeline ----
            def chunk(b, st, u, prefetch=None):
                sl = slice(u * MC, (u + 1) * MC)
                if prefetch is not None:
                    prefetch()
                E = ep.tile([128, NT, MC], BF16, tag="E")
                mem_ps = ps_mem.tile([128, CT, 512], F32, tag="mem")
                out_sb = outp.tile([128, CT, MC], BF16, tag="osb")
                cs_ps = ps_cs.tile([2, 512], F32, tag="cs")
                esA = esump.tile([128, MC], F32R, tag="esA")
                esB = esump.tile([128, MC], F32R, tag="esB")
                rcf = rowp.tile([1, MC], F32, tag="rcf")
                rcr = rowp.tile([1, MC], F32R, tag="rc")

                def mm1(t):
                    n0, pt = _ntile(t)
                    sp = ps_s.tile([128, 512], F32, tag="s")
                    nc.tensor.matmul(sp[0:pt, 0:MC], st["mkb"][:, n0:n0 + pt],
                                     st["qkb"][:, sl], start=True, stop=True)
                    nc.scalar.activation(E[0:pt, t, :], sp[0:pt, 0:MC], EXP,
                                         scale=0.25)

                def csacc(t, start=False, stop=False):
                    n0, pt = _ntile(t)
                    nc.tensor.matmul(cs_ps[0:2, 0:MC], ones16[0:pt, :],
                                     E[0:pt, t, :], start=start, stop=stop)

                def reduce_tiles(dst, t0, t1):
                    src = E[:, t0:t1, :].rearrange("p t m -> p m t")
                    with nc.allow_low_precision(
                            reason="f32r accumulation of 10 positive bf16 "
                                   "exp values; 19-bit mantissa is ample"):
                        nc.vector.tensor_reduce(dst[:], src,
                                                mybir.AxisListType.X,
                                                mybir.AluOpType.add)

                for t in range(NT):
                    mm1(t)
                    if t == NRED + 1:
                        reduce_tiles(esA, 0, NRED)
                    elif t == 2 * NRED + 1:
                        reduce_tiles(esB, NRED, 2 * NRED)
                    if t >= LAG:
                        mm2grp(t - LAG, st, mem_ps, E)
                # drain: mm2 tail with denominator + reciprocal broadcast
                # chains interleaved under PE work
                for i, t in enumerate(range(NT - LAG, NT)):
                    mm2grp(t, st, mem_ps, E)
                    if i == 0:
                        csacc(CSTART, start=True)
                        csacc(CSTART + 1)
                    elif i == 1:
                        csacc(CSTART + 2)
                        csacc(CSTART + 3)
                    elif i == 2:
                        csacc(CSTART + 4)
                    elif i == 3:
                        nc.tensor.matmul(cs_ps[0:2, 0:MC], ones128r[:],
                                         esA[:], start=False, stop=False)
                        nc.tensor.matmul(cs_ps[0:2, 0:MC], ones128r[:],
                                         esB[:], start=False, stop=True)
                    elif i == 4:
                        nc.vector.reciprocal_approx_fast(rcf[:],
                                                         cs_ps[0:1, 0:MC])
                        nc.vector.tensor_copy(rcr[:], rcf[:])
                    elif i == 6:
                        bc_ps = ps_tr.tile([128, 512], F32, tag="tr")
                        nc.tensor.matmul(bc_ps[:, 0:MC], onesb[:], rcr[:],
                                         start=True, stop=True)
                        bc_sb = bcp.tile([128, MC], F32, tag="bc")
                        nc.vector.tensor_copy(bc_sb[:], bc_ps[:, 0:MC])
                # evacuate + normalize on DVE, bf16 out
                for c in range(CT):
                    nc.vector.tensor_mul(out_sb[:, c, :], mem_ps[:, c, 0:MC],
                                         bc_sb[:])
                dst = out_d[b].rearrange("(ct p) n -> p ct n", p=128)
                nc.sync.dma_start(dst[:, :, sl], out_sb[:])

            def body():
                # Batch 0's startup is emitted at the body top (its DMAs
                # still prefetch across the For_i boundary); batch i+1's
                # startup is emitted inside batch i's chunk loop.
                sts = {0: emit_startup(batches[0])}
                for i in range(NBAT):
                    for u in range(CH):
                        pf = None
                        if i + 1 < NBAT:
                            if u == 2:
                                pf = lambda j=i + 1: sts.__setitem__(
                                    j, emit_staging(batches[j]))
                            elif u == 4:
                                pf = lambda j=i + 1: emit_mvt(sts[j])
                        chunk(batches[i], sts[i], u, pf)

            if loop:
                with tc.For_i(0, loop, 1):
                    body()
            else:
                body()

    nc.compile()
    return nc


_NC = None


def _get_nc():
    global _NC
    if _NC is None:
        _NC = build_nc()
    return _NC


def kernel(mk, qk, mv, qv):
    mk = np.ascontiguousarray(np.asarray(mk, dtype=np.float32)).reshape(B, CK, N)
    qk = np.ascontiguousarray(np.asarray(qk, dtype=np.float32)).reshape(B, CK, N)
    mv = np.ascontiguousarray(np.asarray(mv, dtype=np.float32)).reshape(B, CV, N)
    ident = np.eye(128, dtype=np.float32)

    nc = _get_nc()
    in_maps = [
        {"mk": mk[NB * i:NB * (i + 1)],
         "qk": qk[NB * i:NB * (i + 1)],
         "mv": mv[NB * i:NB * (i + 1)],
         "ident": ident}
        for i in range(NCORES)
    ]
    res = run_bass_kernel_spmd(nc, in_maps, list(range(NCORES)))
    mem = np.concatenate([np.asarray(res.results[i]["out"]).astype(np.float32)
                          for i in range(NCORES)], axis=0)
    mem = mem.reshape(B, CV, H, W)
    return mem, np.asarray(qv)
